# revision 5
# baseline (speedup 1.0000x reference)
"""Batched Bjorck orthogonalization on 8 TRN2 NeuronCores.

w: [64, 1024, 1024] f32. 13 iterations of W <- 1.5 W - 0.5 W (W^T W).
Sharding: batch dim across 8 cores (8 matrices per core), fully independent.

Math: single-pass fp32r (e8m11) matmuls on the PE array, 1 cycle/row
(vs 3 for the tf32x3 hi/lo scheme, vs 4 for native fp32). The Bjorck
iteration tolerates the ~2^-11 input rounding: final rel err ~1e-4
vs the 2e-2 gate.

Per-core per-matrix iteration (all on-chip, state in SBUF):
  G: A = 1.5 I - 0.5 (W^T W)   [exact upper-triangle tiles, 4736 of 8192
     output cols; lower blocks reconstructed via PE transposes; the -0.5
     scale folded into the PSUM->SBUF drain; +1.5I added in-place on the
     diagonal blocks by the DVE]
  U: W = W A                   [128 matmuls]
  T: refresh W^T               [64 PE transposes, skipped after last iter;
     dc>=4 first so they overlap U's tail drains]
Modeled per-core device time (InstructionCostModel): ~5.4 ms for 8 matrices.
"""

import numpy as np

_NC_CACHE = {}

P = 128  # partitions
NMAT = 1024  # matrix dim
C = 8  # row chunks (NMAT / P)
FB = 512  # psum free-block width
NB = 2  # free blocks per 1024 (NMAT / FB)
ITERS = 13
PG_BUFS = 4
PU_BUFS = 2
PT_BUFS = 2

# G-phase upper-triangle tile plan: (m, col_start, width). Each tile
# accumulates A[m*128:(m+1)*128, cs:cs+w] over the 8 k-chunks. Widths are
# kept >=256 (fp32r needs N>=256 for 1 cycle/row). m=7 is widened to 256
# (computes lower block (7,6) directly, sparing its recon transpose).
G_TILES = [
    (0, 0, 512), (0, 512, 512),
    (1, 128, 384), (1, 512, 512),
    (2, 256, 256), (2, 512, 512),
    (3, 384, 384), (3, 768, 256),
    (4, 512, 512),
    (5, 640, 384),
    (6, 768, 256),
    (7, 768, 256),
]

# Lower 128x128 blocks not covered above, reconstructed as transposes of
# their upper mirrors; (mb, [contiguous nb run]) ordered by source
# availability (sources live in chunk nb, computed in m order).
G_RECON = [
    (1, [0]),
    (2, [0, 1]),
    (3, [0, 1, 2]),
    (4, [0, 1, 2, 3]),
    (5, [0, 1, 2, 3]),
    (6, [0, 1, 2, 3]),
    (7, [0, 1, 2, 3]),
    (5, [4]),
    (6, [4, 5]),
    (7, [4, 5]),
]


def _build(B, iters=ITERS, static=False):
    import concourse.bacc as bacc
    import concourse.bass as bass
    import concourse.mybir as mybir
    from concourse.tile import TileContext

    F32 = mybir.dt.float32
    F32R = mybir.dt.float32r
    COPY = mybir.ActivationFunctionType.Copy
    ADD = mybir.AluOpType.add

    nc = bacc.Bacc("TRN2", target_bir_lowering=False, debug=False)
    w = nc.dram_tensor("w", [B, NMAT, NMAT], F32, kind="ExternalInput")
    o = nc.dram_tensor("o", [B, NMAT, NMAT], F32, kind="ExternalOutput")
    eye = nc.dram_tensor("eye", [P, P], F32, kind="ExternalInput")  # 1.5*I
    ide = nc.dram_tensor("ide", [P, P], F32, kind="ExternalInput")  # I

    with TileContext(nc) as tc:
        with (
            tc.tile_pool(name="state", bufs=1) as st,
            tc.tile_pool(name="const", bufs=1) as cn,
            tc.tile_pool(name="tmp", bufs=3) as tp,
            tc.tile_pool(name="pg", bufs=PG_BUFS, space="PSUM") as pg,
            tc.tile_pool(name="pu", bufs=PU_BUFS, space="PSUM") as pu,
            tc.tile_pool(name="pt", bufs=PT_BUFS, space="PSUM") as pt,
        ):
            eye_s = cn.tile([P, P], F32, tag="eye")
            ide32 = cn.tile([P, P], F32, tag="ide32")
            ide_r = cn.tile([P, P], F32R, tag="ide_r")
            nc.sync.dma_start(eye_s[:], eye.ap())
            nc.sync.dma_start(ide32[:], ide.ap())
            nc.scalar.activation(ide_r[:], ide32[:], COPY)

            from contextlib import nullcontext

            loop_cm = nullcontext(0) if static else tc.For_i(0, B)
            with loop_cm as ib:
              for _sib in range(B if static else 1):
                if static:
                    ib = _sib
                W = st.tile([P, C * NMAT], F32R, tag="W")
                WT = st.tile([P, C * NMAT], F32R, tag="WT")
                A = st.tile([P, C * NMAT], F32R, tag="A")

                # ---- load W: DMA fp32 into tmp, Act-round into the f32r tile
                # (the BIR verifier requires f32r matmul inputs to be written
                # by a rounding engine op, not raw DMA bits)
                for c in range(C):
                    for nb2 in range(NB):
                        t32 = tp.tile([P, FB], F32, tag="t32")
                        nc.sync.dma_start(
                            t32[:],
                            w.ap()[
                                bass.ds(ib, 1),
                                c * P : (c + 1) * P,
                                nb2 * FB : (nb2 + 1) * FB,
                            ],
                        )
                        nc.scalar.activation(
                            W[:, c * NMAT + nb2 * FB : c * NMAT + (nb2 + 1) * FB],
                            t32[:],
                            COPY,
                        )

                def phase_T():
                    # WT[dc] = transpose of column-block dc of W; dc>=4 first
                    # (their U-phase source drains complete earliest)
                    for dc in (4, 5, 6, 7, 0, 1, 2, 3):
                        for half in range(NB):
                            ptile = pt.tile([P, FB], F32R, tag="pt")
                            for q in range(4):
                                i = half * 4 + q  # source row-chunk
                                nc.tensor.transpose(
                                    ptile[:, q * P : (q + 1) * P],
                                    W[:, i * NMAT + dc * P : i * NMAT + (dc + 1) * P],
                                    ide_r[:],
                                )
                            nc.scalar.activation(
                                WT[:, dc * NMAT + half * FB : dc * NMAT + (half + 1) * FB],
                                ptile[:],
                                COPY,
                            )

                phase_T()

                for it in range(iters):
                    last = it == iters - 1
                    # ---- G: A = 1.5 I - 0.5 W^T W (upper triangle)
                    for m, cs, wd in G_TILES:
                        g = pg.tile([P, FB], F32, tag="pg")
                        for k in range(C):
                            nc.tensor.matmul(
                                g[:, :wd],
                                W[:, k * NMAT + m * P : k * NMAT + (m + 1) * P],
                                W[:, k * NMAT + cs : k * NMAT + cs + wd],
                                start=(k == 0),
                                stop=(k == C - 1),
                            )
                        if cs <= m * P < cs + wd:
                            # diag tile: detour through f32 tmp for the +1.5I
                            # (DVE output can't feed f32r matmuls directly)
                            doff = m * P - cs
                            t32 = tp.tile([P, FB], F32, tag="t32")
                            nc.scalar.activation(
                                t32[:, :wd], g[:, :wd], COPY, scale=-0.5
                            )
                            nc.vector.tensor_tensor(
                                t32[:, doff : doff + P],
                                t32[:, doff : doff + P],
                                eye_s[:],
                                ADD,
                            )
                            nc.scalar.activation(
                                A[:, m * NMAT + cs : m * NMAT + cs + wd],
                                t32[:, :wd],
                                COPY,
                            )
                        else:
                            nc.scalar.activation(
                                A[:, m * NMAT + cs : m * NMAT + cs + wd],
                                g[:, :wd],
                                COPY,
                                scale=-0.5,
                            )
                    # ---- G recon: lower blocks = transpose of upper mirrors
                    for mb, nbs in G_RECON:
                        n_r = len(nbs) * P
                        pr = pt.tile([P, FB], F32R, tag="pt")
                        for qi, nb in enumerate(nbs):
                            nc.tensor.transpose(
                                pr[:, qi * P : (qi + 1) * P],
                                A[:, nb * NMAT + mb * P : nb * NMAT + (mb + 1) * P],
                                ide_r[:],
                            )
                        nc.scalar.activation(
                            A[:, mb * NMAT + nbs[0] * P : mb * NMAT + nbs[0] * P + n_r],
                            pr[:, :n_r],
                            COPY,
                        )
                    # ---- U: W = W A
                    for nb2 in (1, 0):
                        for i in range(C):
                            u = pu.tile([P, FB], F32, tag="pu")
                            for j in range(C):
                                nc.tensor.matmul(
                                    u[:],
                                    WT[:, j * NMAT + i * P : j * NMAT + (i + 1) * P],
                                    A[:, j * NMAT + nb2 * FB : j * NMAT + (nb2 + 1) * FB],
                                    start=(j == 0),
                                    stop=(j == C - 1),
                                )
                            if last:
                                t32o = tp.tile([P, FB], F32, tag="t32")
                                nc.scalar.activation(t32o[:], u[:], COPY)
                                nc.sync.dma_start(
                                    o.ap()[
                                        bass.ds(ib, 1),
                                        i * P : (i + 1) * P,
                                        nb2 * FB : (nb2 + 1) * FB,
                                    ],
                                    t32o[:],
                                )
                            else:
                                nc.scalar.activation(
                                    W[:, i * NMAT + nb2 * FB : i * NMAT + (nb2 + 1) * FB],
                                    u[:],
                                    COPY,
                                )
                    if not last:
                        phase_T()
    nc.compile()
    return nc


def _get_nc(B, iters=ITERS):
    key = (B, iters)
    if key not in _NC_CACHE:
        _NC_CACHE[key] = _build(B, iters)
    return _NC_CACHE[key]


def kernel(w) -> np.ndarray:
    from concourse.bass_utils import run_bass_kernel_spmd

    w = np.ascontiguousarray(np.asarray(w, dtype=np.float32))
    assert w.shape == (64, NMAT, NMAT), w.shape
    B = 8  # matrices per core
    nc = _get_nc(B)
    eye15 = (1.5 * np.eye(P)).astype(np.float32)
    ide = np.eye(P, dtype=np.float32)
    in_maps = [
        {"w": np.ascontiguousarray(w[c * B : (c + 1) * B]), "eye": eye15, "ide": ide}
        for c in range(8)
    ]
    res = run_bass_kernel_spmd(nc, in_maps, core_ids=list(range(8)))
    return np.concatenate([res.results[c]["o"] for c in range(8)], axis=0)


# revision 17
# speedup vs baseline: 1.1380x; 1.1380x over previous
"""Batched Bjorck orthogonalization on 8 TRN2 NeuronCores.

w: [64, 1024, 1024] f32. 13 iterations of W <- 1.5 W - 0.5 W (W^T W).
Sharding: batch dim across 8 cores (8 matrices per core), fully independent.

Math: single-pass fp16 (e5m10) matmuls on the PE array, 1 cycle/row
(vs 3x fp32r for the tf32x3 hi/lo scheme, vs 4 for native fp32), with
fp32 PSUM accumulation. The Bjorck iteration tolerates the ~2^-10 input
rounding: final rel err ~2e-3 vs the 2e-2 gate. fp16 also makes PE
transposes 1.0 cycles/row (fp32r pays 1.5) and drops the fp32r N>=256
matmul-width constraint, so the Gram triangle is exact.

Per-core per-matrix iteration (all on-chip, state in SBUF):
  G: A = 1.5 I - 0.5 (W^T W)   [exact upper-triangle tiles, 4608 of 8192
     output cols; lower blocks reconstructed via PE transposes; the -0.5
     scale folded into the PSUM->SBUF drain; +1.5I added in place on the
     diagonal blocks by the (otherwise idle) DVE]
  U: W = W A                   [128 matmuls]
  T: refresh W^T               [64 PE transposes, skipped after last iter;
     dc>=4 first so they overlap U's tail drains]
Engine split: PE does matmuls+transposes (the critical path, ~114.2k
cycles/iter/matrix); Act drains G tiles + U tiles + half the loads; DVE
drains recon + T transposes and patches diagonals; Pool covers the other
half of the loads so they never queue behind output drains.
Modeled per-core device time (InstructionCostModel): ~5.06 ms for 8
matrices.
"""

import numpy as np

_NC_CACHE = {}

P = 128  # partitions
NMAT = 1024  # matrix dim
C = 8  # row chunks (NMAT / P)
FB = 512  # psum free-block width
NB = 2  # free blocks per 1024 (NMAT / FB)
ITERS = 13
PG_BUFS = 3
PU_BUFS = 2
PT_BUFS = 3

# G-phase upper-triangle tile plan: (m, col_start, width). Each tile
# accumulates A[m*128:(m+1)*128, cs:cs+w] over the 8 k-chunks.
G_TILES = [
    (0, 0, 512), (0, 512, 512),
    (1, 128, 384), (1, 512, 512),
    (2, 256, 256), (2, 512, 512),
    (3, 384, 384), (3, 768, 256),
    (4, 512, 512),
    (5, 640, 384),
    (6, 768, 256),
    (7, 896, 128),
]

# Lower 128x128 blocks not covered above, reconstructed as transposes of
# their upper mirrors; (mb, [contiguous nb run]). The nb>=4 runs go first:
# the U phase consumes the nb2=1 column half first, and their sources
# (G tiles m=4..6) drain before the rest of the phase completes.
G_RECON = [
    (5, [4]),
    (6, [4, 5]),
    (7, [4, 5]),
    (7, [6]),
    (1, [0]),
    (2, [0, 1]),
    (3, [0, 1, 2]),
    (4, [0, 1, 2, 3]),
    (5, [0, 1, 2, 3]),
    (6, [0, 1, 2, 3]),
    (7, [0, 1, 2, 3]),
]


def _build(B, iters=ITERS, static=False):
    import concourse.bacc as bacc
    import concourse.bass as bass
    import concourse.mybir as mybir
    from concourse.tile import TileContext

    F32 = mybir.dt.float32
    F16 = mybir.dt.float16
    COPY = mybir.ActivationFunctionType.Copy
    ADD = mybir.AluOpType.add

    # process matrices in pairs, phase-interleaved: the PE gets ~15us of
    # independent work from the partner matrix across every cross-engine
    # dependency (PSUM drain, recon source, load copy), hiding those stalls
    PAIR = 2 if B % 2 == 0 else 1
    NPAIR = B // PAIR

    nc = bacc.Bacc("TRN2", target_bir_lowering=False, debug=False)
    w = nc.dram_tensor("w", [NPAIR, PAIR, NMAT, NMAT], F32, kind="ExternalInput")
    o = nc.dram_tensor("o", [NPAIR, PAIR, NMAT, NMAT], F32, kind="ExternalOutput")
    eye = nc.dram_tensor("eye", [P, P], F32, kind="ExternalInput")  # 1.5*I
    ide = nc.dram_tensor("ide", [P, P], F32, kind="ExternalInput")  # I

    with TileContext(nc) as tc:
        with (
            tc.tile_pool(name="state", bufs=1) as st,
            tc.tile_pool(name="const", bufs=1) as cn,
            tc.tile_pool(name="tmp", bufs=3) as tp,
            tc.tile_pool(name="pg", bufs=PG_BUFS, space="PSUM") as pg,
            tc.tile_pool(name="pu", bufs=PU_BUFS, space="PSUM") as pu,
            tc.tile_pool(name="pt", bufs=PT_BUFS, space="PSUM") as pt,
        ):
            eye32 = cn.tile([P, P], F32, tag="eye32")
            eye16 = cn.tile([P, P], F16, tag="eye16")
            ide32 = cn.tile([P, P], F32, tag="ide32")
            ide16 = cn.tile([P, P], F16, tag="ide16")
            nc.sync.dma_start(eye32[:], eye.ap())
            nc.sync.dma_start(ide32[:], ide.ap())
            nc.scalar.activation(eye16[:], eye32[:], COPY)
            nc.scalar.activation(ide16[:], ide32[:], COPY)

            from contextlib import nullcontext

            def load(ib, s, W, nb2):
                for c in range(C):
                    tl = tp.tile([P, FB], F32, tag="tl")
                    nc.sync.dma_start(
                        tl[:],
                        w.ap()[
                            bass.ds(ib, 1),
                            s,
                            c * P : (c + 1) * P,
                            nb2 * FB : (nb2 + 1) * FB,
                        ],
                    )
                    dst = W[:, c * NMAT + nb2 * FB : c * NMAT + (nb2 + 1) * FB]
                    # split across Act and the idle Pool engine so loads
                    # don't queue behind the previous pair's output drains
                    if c % 2 == 0:
                        nc.scalar.activation(dst, tl[:], COPY)
                    else:
                        nc.gpsimd.tensor_copy(dst, tl[:])

            def phase_T(W, WT):
                # WT[dc] = transpose of column-block dc of W; dc>=4 first
                # (their U-phase source drains complete earliest).
                # Drains on DVE to keep Act free for G/U drains.
                for dc in (4, 5, 6, 7, 0, 1, 2, 3):
                    for half in range(NB):
                        ptile = pt.tile([P, FB], F16, tag="pt")
                        for q in range(4):
                            i = half * 4 + q  # source row-chunk
                            nc.tensor.transpose(
                                ptile[:, q * P : (q + 1) * P],
                                W[:, i * NMAT + dc * P : i * NMAT + (dc + 1) * P],
                                ide16[:],
                            )
                        nc.vector.tensor_copy(
                            WT[:, dc * NMAT + half * FB : dc * NMAT + (half + 1) * FB],
                            ptile[:],
                        )

            def phase_G(W, A):
                # A = 1.5 I - 0.5 W^T W (upper triangle)
                for m, cs, wd in G_TILES:
                    g = pg.tile([P, FB], F32, tag="pg")
                    for k in range(C):
                        nc.tensor.matmul(
                            g[:, :wd],
                            W[:, k * NMAT + m * P : k * NMAT + (m + 1) * P],
                            W[:, k * NMAT + cs : k * NMAT + cs + wd],
                            start=(k == 0),
                            stop=(k == C - 1),
                        )
                    nc.scalar.activation(
                        A[:, m * NMAT + cs : m * NMAT + cs + wd],
                        g[:, :wd],
                        COPY,
                        scale=-0.5,
                    )
                    if cs <= m * P < cs + wd:  # diag block: += 1.5 I (DVE)
                        d = m * NMAT + m * P
                        nc.vector.tensor_tensor(
                            A[:, d : d + P], A[:, d : d + P], eye16[:], ADD
                        )

            def phase_recon(A):
                # lower blocks = transpose of upper mirrors
                for mb, nbs in G_RECON:
                    n_r = len(nbs) * P
                    pr = pt.tile([P, FB], F16, tag="pt")
                    for qi, nb in enumerate(nbs):
                        nc.tensor.transpose(
                            pr[:, qi * P : (qi + 1) * P],
                            A[:, nb * NMAT + mb * P : nb * NMAT + (mb + 1) * P],
                            ide16[:],
                        )
                    nc.vector.tensor_copy(
                        A[:, mb * NMAT + nbs[0] * P : mb * NMAT + nbs[0] * P + n_r],
                        pr[:, :n_r],
                    )

            def phase_U(ib, s, W, WT, A, last):
                # W = W A
                for nb2 in (1, 0):
                    for i in range(C):
                        u = pu.tile([P, FB], F32, tag="pu")
                        for j in range(C):
                            nc.tensor.matmul(
                                u[:],
                                WT[:, j * NMAT + i * P : j * NMAT + (i + 1) * P],
                                A[:, j * NMAT + nb2 * FB : j * NMAT + (nb2 + 1) * FB],
                                start=(j == 0),
                                stop=(j == C - 1),
                            )
                        if last:
                            t32o = tp.tile([P, FB], F32, tag="t32")
                            nc.scalar.activation(t32o[:], u[:], COPY)
                            nc.sync.dma_start(
                                o.ap()[
                                    bass.ds(ib, 1),
                                    s,
                                    i * P : (i + 1) * P,
                                    nb2 * FB : (nb2 + 1) * FB,
                                ],
                                t32o[:],
                            )
                        else:
                            nc.scalar.activation(
                                W[:, i * NMAT + nb2 * FB : i * NMAT + (nb2 + 1) * FB],
                                u[:],
                                COPY,
                            )

            loop_cm = nullcontext(0) if static else tc.For_i(0, NPAIR)
            with loop_cm as ib:
              for _sib in range(NPAIR if static else 1):
                if static:
                    ib = _sib
                Ws, WTs, As = [], [], []
                for s in range(PAIR):
                    Ws.append(st.tile([P, C * NMAT], F16, tag=f"W{s}", name=f"W{s}"))
                    WTs.append(st.tile([P, C * NMAT], F16, tag=f"WT{s}", name=f"WT{s}"))
                    As.append(st.tile([P, C * NMAT], F16, tag=f"A{s}", name=f"A{s}"))

                # loads: nb2=1 halves first (initial T dc>=4 needs only those)
                for s in range(PAIR):
                    load(ib, s, Ws[s], 1)
                for s in range(PAIR):
                    load(ib, s, Ws[s], 0)
                for s in range(PAIR):
                    phase_T(Ws[s], WTs[s])

                for it in range(iters):
                    last = it == iters - 1
                    for s in range(PAIR):
                        phase_G(Ws[s], As[s])
                    for s in range(PAIR):
                        phase_recon(As[s])
                    for s in range(PAIR):
                        phase_U(ib, s, Ws[s], WTs[s], As[s], last)
                    if not last:
                        for s in range(PAIR):
                            phase_T(Ws[s], WTs[s])
    nc.compile()
    return nc


def _get_nc(B, iters=ITERS):
    key = (B, iters)
    if key not in _NC_CACHE:
        _NC_CACHE[key] = _build(B, iters)
    return _NC_CACHE[key]


def kernel(w) -> np.ndarray:
    from concourse.bass_utils import run_bass_kernel_spmd

    w = np.ascontiguousarray(np.asarray(w, dtype=np.float32))
    assert w.shape == (64, NMAT, NMAT), w.shape
    B = 8  # matrices per core
    nc = _get_nc(B)
    eye15 = (1.5 * np.eye(P)).astype(np.float32)
    ide = np.eye(P, dtype=np.float32)
    in_maps = [
        {
            "w": np.ascontiguousarray(w[c * B : (c + 1) * B]).reshape(
                B // 2, 2, NMAT, NMAT
            ),
            "eye": eye15,
            "ide": ide,
        }
        for c in range(8)
    ]
    res = run_bass_kernel_spmd(nc, in_maps, core_ids=list(range(8)))
    return np.concatenate(
        [res.results[c]["o"].reshape(B, NMAT, NMAT) for c in range(8)], axis=0
    )
